# revision 17
# baseline (speedup 1.0000x reference)
"""Trainium2 Bass kernel for nn_Attention_53455162966555.

Multi-head attention block: B=8, N=1024, DIM=1024, H=16 heads, hd=64.
Sharding: data-parallel over batch — core b computes x[b] with full weights
on NeuronCore b; no collectives.

Precision/speed strategy (per the TimelineSim cost model, fp8e4 DoubleRow
matmuls run at 0.5 cycles/row with a 2x128 contraction — 4x f32r/bf16
FLOPs/cycle — but single-fp8 operands are too noisy for the concentrated
softmax rows of this data, so every fp8 matmul here carries hi/lo pairs):

  - x^T and 16*w_qkv are split on the host into fp8e4 hi + lo residual
    tensors (hi = fp8(t), lo = fp8(t - hi) — ~11 effective mantissa bits).
  - qkv projections: 3 DoubleRow passes (hi.hi, lo.hi, hi.lo) with the
    DR pair-dim carrying contraction-tile pairs: 0.75 c/row equivalent.
  - scores: q^T/k^T PSUM results are re-split into fp8 hi/lo (DVE), then
    assembled by SBUF->SBUF DMAs into stacked layouts: Q* = [qh; ql] on
    128 partitions, K* duplicated per half with k-hi/lo in the DR pair
    dim. One DR matmul per (head, kt, qc) then computes the full bilinear
    (qh+ql).(kh+kl) = q.k — exact to hi/lo precision at 0.5 c/row with
    contraction 2x128 (the fp8 q/k quantization error, ~0.1%, is far
    below what single fp8 gives). rhs uses a stride-0 broadcast pair dim.
  - exp on ScalarE from PSUM with scale=1/2048 (undoes the 16x16 weight
    scaling and applies 1/sqrt(hd)) and bias=-4 (scores reach 7.74 on
    this data; e4m3 infs above 240 — a constant bias cancels exactly in
    the softmax normalization), writing bf16.
  - P.V in bf16 (p cannot be fp8: its quantization alone costs ~1.9e-2
    max-rel error on the hot rows, right at the 2e-2 gate).
  - normalization via the 16.0 ones-column denominator row: DVE
    reciprocal -> GpSimd partition_broadcast -> DVE multiply -> bf16 O^T.
  - output projection in bf16 (w_proj row-permuted + bf16-cast on host).
    Contraction split cs 0-4 (emitted into attention slots during head
    pairs 5-6, partials to SBUF) + cs 5-7 tail, to keep PE busy end-to-end.

Engine budget: PE ~171us (wall), ScalarE (128 exps) ~133us, DVE ~110us.
"""

import numpy as np
import ml_dtypes

import concourse.bass as bass
import concourse.mybir as mybir
import concourse.tile as tile
from concourse import bacc

P = 128
DIM = 1024
H = 16
HD = 64
F3 = 3 * DIM
CS = DIM // P
QC = 512

FP32 = mybir.dt.float32
FP32R = mybir.dt.float32r
FP8 = mybir.dt.float8e4
BF16 = mybir.dt.bfloat16
Exp = mybir.ActivationFunctionType.Exp
DR = mybir.MatmulPerfMode.DoubleRow

F8NP = ml_dtypes.float8_e4m3
BF16NP = ml_dtypes.bfloat16

EXP_SCALE = (HD ** -0.5) / 256.0
EXP_BIAS = -4.0


def build_nc(N=1024):
    NT = N // P
    NQ = N // QC

    nc = bacc.Bacc(None, target_bir_lowering=False)
    with tile.TileContext(nc) as tc:
        with tc.tile_pool(name="dram", bufs=1, space="DRAM") as dram:
            xh_d = dram.tile([DIM, N], FP8, kind="ExternalInput")
            xl_d = dram.tile([DIM, N], FP8, kind="ExternalInput")
            wh_d = dram.tile([DIM, F3], FP8, kind="ExternalInput")
            wl_d = dram.tile([DIM, F3], FP8, kind="ExternalInput")
            wp_d = dram.tile([DIM, DIM], BF16, kind="ExternalInput")
            y_d = dram.tile([N, DIM], FP32, kind="ExternalOutput")
            _build_core(nc, tc, xh_d, xl_d, wh_d, wl_d, wp_d, y_d, N, NT, NQ)
    nc.compile()
    names = dict(xh=xh_d.name, xl=xl_d.name, wh=wh_d.name, wl=wl_d.name,
                 wp=wp_d.name, y=y_d.name)
    return nc, names


def _build_core(nc, tc, xh_d, xl_d, wh_d, wl_d, wp_d, y_d, N, NT, NQ):
    xh_r = xh_d[:].rearrange("(cs p) n -> p cs n", p=P)
    xl_r = xl_d[:].rearrange("(cs p) n -> p cs n", p=P)
    wh_r = wh_d[:].rearrange("(cs p) f -> p cs f", p=P)
    wl_r = wl_d[:].rearrange("(cs p) f -> p cs f", p=P)
    wp_r = wp_d[:].rearrange("(cs p) f -> p cs f", p=P)
    y_r = y_d[:].rearrange("(nt p) f -> p nt f", p=P)

    with (
        tc.tile_pool(name="consts", bufs=1) as consts,
        tc.tile_pool(name="persist", bufs=1) as persist,
        tc.tile_pool(name="wqs", bufs=2) as wqs_pool,
        tc.tile_pool(name="qstar", bufs=2) as qstar_pool,
        tc.tile_pool(name="kstar", bufs=2) as kstar_pool,
        tc.tile_pool(name="est", bufs=3) as est_pool,
        tc.tile_pool(name="recip", bufs=2) as recip_pool,
        tc.tile_pool(name="rb", bufs=2) as rb_pool,
        tc.tile_pool(name="ysb", bufs=2) as ysb_pool,
        tc.tile_pool(name="psum", bufs=1, space="PSUM") as psum,
    ):
        bias_t = consts.tile([P, 1], FP32)
        nc.gpsimd.memset(bias_t[:], EXP_BIAS)

        xTh = persist.tile([P, CS, N], FP8)
        xTl = persist.tile([P, CS, N], FP8)
        wvh = persist.tile([P, CS, DIM], FP8)
        wvl = persist.tile([P, CS, DIM], FP8)
        # q/k hi/lo staging: t[q|k]8[p, a(hi/lo), hp, n], p = head-parity*64+d
        tq8 = persist.tile([P, 2, CS, N], FP8)
        tk8 = persist.tile([P, 2, CS, N], FP8)
        V_sb = persist.tile([P, NT, H, HD + 1], BF16)
        OT = persist.tile([P, CS, N], BF16)
        wpb = persist.tile([P, CS, DIM], BF16)
        y1 = persist.tile([P, NT, DIM], FP32)

        vones = consts.tile([P, NT, H, 1], BF16)
        nc.gpsimd.memset(vones[:], 16.0)
        nc.gpsimd.tensor_copy(V_sb[:, :, :, HD:HD + 1], vones[:])


        # ---- 3-pass hi/lo fp8 DoubleRow projection helper -----------------
        def dr3(out_ap, lhs_pairs, rhs_pairs):
            # lhs_pairs/rhs_pairs: (hi_tile_slice_fn, lo_tile_slice_fn)
            combos = ((0, 0), (1, 0), (0, 1))  # (x sel, w sel): hh, lh, hl
            n = 0
            for (ia, ib) in combos:
                for j in range(4):
                    nc.tensor.matmul(
                        out_ap,
                        lhs_pairs[ia](j),
                        rhs_pairs[ib](j),
                        start=(n == 0), stop=(n == 11), perf_mode=DR,
                    )
                    n += 1

        # ---- V' = x @ (16 Wv), hi/lo fp8 DR, out bf16 ---------------------
        def emit_vproj(nt, fc):
            pv = psum.tile([P, QC], FP32, tag="u", bufs=2,
                           name=f"pv_{nt}_{fc}")
            xs = lambda t: (lambda j: t[:, 2 * j:2 * j + 2, nt * P:(nt + 1) * P])
            ws = lambda t: (lambda j: t[:, 2 * j:2 * j + 2,
                                        fc * QC:(fc + 1) * QC])
            dr3(pv[:], (xs(xTh), xs(xTl)), (ws(wvh), ws(wvl)))
            nc.vector.tensor_copy(
                V_sb[:, nt, fc * 8:(fc + 1) * 8, 0:HD],
                pv[:, :].rearrange("p (h d) -> p h d", d=HD),
            )

        # ---- q/k proj (hi/lo DR) -> fp8 hi/lo -> Q*/K* assembly -----------
        def load_wq(hp, ft, eng, eng_lo=None):
            wqh = wqs_pool.tile([P, CS, P], FP8, tag="wqs",
                                name=f"wqh_{hp}_{ft}")
            wql = wqs_pool.tile([P, CS, P], FP8, tag="wql",
                                name=f"wql_{hp}_{ft}")
            eng.dma_start(wqh[:], wh_r[:, :, ft * P:(ft + 1) * P])
            (eng_lo or eng).dma_start(wql[:], wl_r[:, :, ft * P:(ft + 1) * P])
            return wqh, wql

        def emit_qk_proj0(preloaded):
            # hp0 startup path: skip the hi/lo re-split + stacked assembly;
            # copy q^T/k^T to f32r and run hp0's S in f32r (tile_position
            # row-packed). Shortens the chain to the first exp by ~4us for
            # +8192 PE cycles on this head pair only.
            qk0 = qstar_pool.tile([P, 2, N], FP32R, tag="qk0f32r",
                                  name="qk_t0")
            for ti, (t8, ft) in enumerate(((tq8, 0), (tk8, CS))):
                wqh, wql = preloaded[ti]
                for qc in range(NQ):
                    pqk = psum.tile([P, QC], FP32, tag="u", bufs=2,
                                    name=f"pqk0_{ft}_{qc}")
                    xs = lambda t: (lambda j: t[:, 2 * j:2 * j + 2,
                                                qc * QC:(qc + 1) * QC])
                    ws = lambda t: (lambda j: t[:, 2 * j:2 * j + 2, :])
                    dr3(pqk[:], (ws(wqh), ws(wql)), (xs(xTh), xs(xTl)))
                    nc.vector.tensor_copy(
                        qk0[:, ti, qc * QC:(qc + 1) * QC], pqk[:])
            return qk0

        def emit_qk_proj(hp, preloaded=None):
            qs = qstar_pool.tile([P, 2, N], FP8, tag="qstar",
                                 name=f"qstar_{hp}")
            ks = kstar_pool.tile([P, 2, 2, N], FP8, tag="kstar",
                                 name=f"kstar_{hp}")
            for ti, (t8, ft) in enumerate(((tq8, hp), (tk8, CS + hp))):
                if preloaded is not None:
                    wqh, wql = preloaded[ti]
                else:
                    wqh, wql = load_wq(hp, ft, nc.sync)
                for qc in range(NQ):
                    pqk = psum.tile([P, QC], FP32, tag="u", bufs=2,
                                    name=f"pqk_{hp}_{ft}_{qc}")
                    xs = lambda t: (lambda j: t[:, 2 * j:2 * j + 2,
                                                qc * QC:(qc + 1) * QC])
                    ws = lambda t: (lambda j: t[:, 2 * j:2 * j + 2, :])
                    dr3(pqk[:], (ws(wqh), ws(wql)), (xs(xTh), xs(xTl)))
                    sl = slice(qc * QC, (qc + 1) * QC)
                    nc.vector.tensor_copy(t8[:, 0, hp, sl], pqk[:])
                    nc.vector.tensor_sub(t8[:, 1, hp, sl], pqk[:],
                                         t8[:, 0, hp, sl])
            # assemble stacked layouts (partition-base-offset local DMAs):
            #   Q*[a*64+d, e, n] = q_a[head 2hp+e][d, n]
            #   K*[a*64+d, e, i, n] = k_i[head 2hp+e][d, n]  (a-duplicated)
            for a in range(2):
                pa = slice(a * HD, (a + 1) * HD)
                for e in range(2):
                    pe = slice(e * HD, (e + 1) * HD)
                    nc.sync.dma_start(qs[pa, e, :], tq8[pe, a, hp, :])
                    nc.sync.dma_start(ks[pa, e, :, :], tk8[pe, :, hp, :])
            return qs, ks

        # ---- deferred per-slot PE work ------------------------------------
        def emit_proj1(nt, fc):
            py = psum.tile([P, QC], FP32, tag="u", bufs=2,
                           name=f"py1_{nt}_{fc}")
            for cs in range(5):
                nc.tensor.matmul(
                    py[:],
                    OT[:, cs, nt * P:(nt + 1) * P],
                    wpb[:, cs, fc * QC:(fc + 1) * QC],
                    start=(cs == 0), stop=(cs == 4),
                )
            nc.vector.tensor_copy(y1[:, nt, fc * QC:(fc + 1) * QC], py[:])

        # startup order: hp0's small wq loads lead the Act HWDGE queue,
        # x^T quarters stream on both queues right behind, then the hp0
        # projection (the critical path to the first exp), then w_v.
        wq0 = [load_wq(0, 0, nc.sync, nc.scalar),
               load_wq(0, CS, nc.sync, nc.scalar)]
        for qt in range(4):
            s = slice(2 * qt, 2 * (qt + 1))
            nc.sync.dma_start(xTh[:, s, :], xh_r[:, s, :])
            nc.scalar.dma_start(xTl[:, s, :], xl_r[:, s, :])
        qk_next = emit_qk_proj0(wq0)

        # w_v loads follow the hp0 wq loads in queue order; quarter-DMAs so
        # the first V chunks (which only need cs 0..1) start early.
        for qt in range(4):
            s = slice(2 * qt, 2 * (qt + 1))
            nc.sync.dma_start(wvh[:, s, :], wh_r[:, s, 2 * DIM:3 * DIM])
            nc.sync.dma_start(wvl[:, s, :], wl_r[:, s, 2 * DIM:3 * DIM])

        # V chunks for (nt 0,1, fc 0) are needed by the first PV pair;
        # the rest stream through the attention slots (popped 2 per
        # j-block, ahead of the PV that reads them).
        emit_vproj(0, 0)
        emit_vproj(1, 0)
        emit_vproj(2, 0)
        emit_vproj(3, 0)
        slot_work = [(lambda nt=nt: emit_vproj(nt, 0)) for nt in range(4, NT)]
        slot_work += [(lambda nt=nt: emit_vproj(nt, 1)) for nt in range(NT)]

        def emit_tail_part1(nt):
            for fc in range(2):
                py = psum.tile([P, QC], FP32, tag="u", bufs=2,
                               name=f"pyp1_{nt}_{fc}")
                for cs in (5, 6):
                    nc.tensor.matmul(
                        py[:],
                        OT[:, cs, nt * P:(nt + 1) * P],
                        wpb[:, cs, fc * QC:(fc + 1) * QC],
                        start=(cs == 5), stop=(cs == 6),
                    )
                nc.vector.tensor_add(
                    y1[:, nt, fc * QC:(fc + 1) * QC], py[:],
                    y1[:, nt, fc * QC:(fc + 1) * QC])

        def emit_tail(nt):
            y_sb = ysb_pool.tile([P, DIM], FP32, tag="ysb",
                                 name=f"y_sb_{nt}")
            for fc in range(2):
                py2 = psum.tile([P, QC], FP32, tag="u", bufs=2,
                                name=f"py2_{nt}_{fc}")
                for cs in range(5, CS):
                    nc.tensor.matmul(
                        py2[:],
                        OT[:, cs, nt * P:(nt + 1) * P],
                        wpb[:, cs, fc * QC:(fc + 1) * QC],
                        start=(cs == 5), stop=(cs == CS - 1),
                    )
                nc.vector.tensor_add(
                    y_sb[:, fc * QC:(fc + 1) * QC], py2[:],
                    y1[:, nt, fc * QC:(fc + 1) * QC])
            nc.scalar.dma_start(y_r[:, nt, :], y_sb[:])
        for hp in range(CS):
            qkop = qk_next
            if hp == 5:
                slot_work.extend(
                    (lambda nt=nt, fc=fc: emit_proj1(nt, fc))
                    for nt in range(NT) for fc in range(2))
            for qc in range(NQ):
                if hp == CS - 1 and qc == 0:
                    # cs 5-6 partials for the back half overlap hp7/qc0
                    slot_work.extend(
                        (lambda nt=nt: emit_tail_part1(nt))
                        for nt in range(4, NT))
                if hp == CS - 1 and qc == 1:
                    # n rows 0..511 of the o-proj tail only need OT columns
                    # written by qc=0 normalizes — overlap them with the
                    # last head-pair's qc=1 attention
                    slot_work.extend(
                        (lambda nt=nt: emit_tail(nt)) for nt in range(4))
                pacc2 = [psum.tile([HD + 1, QC], FP32, tag="oacc", bufs=2,
                                   name=f"pacc_{hp}_{qc}_{e}")
                         for e in range(2)]
                for j in range(4):
                    if slot_work:
                        slot_work.pop(0)()
                    for e in range(2):
                        h = 2 * hp + e
                        stage = psum.tile([P, 2, QC], FP32, tag="sstage",
                                          bufs=2, name=f"st_{hp}_{qc}_{j}_{e}")
                        if hp == 0:
                            qk0 = qkop
                            po = e * HD
                            for ki in range(2):
                                kt = 2 * j + ki
                                nc.tensor.matmul(
                                    stage[:, ki, :],
                                    qk0[po:po + HD, 1, kt * P:(kt + 1) * P],
                                    qk0[po:po + HD, 0, qc * QC:(qc + 1) * QC],
                                    start=True, stop=True,
                                    tile_position=(po, 0),
                                )
                        else:
                            qs, ks = qkop
                            rhs = qs[:, e, qc * QC:(qc + 1) * QC]
                            rhs = rhs[:, None, :].to_broadcast([P, 2, QC])
                            for ki in range(2):
                                kt = 2 * j + ki
                                nc.tensor.matmul(
                                    stage[:, ki, :],
                                    ks[:, e, :, kt * P:(kt + 1) * P],
                                    rhs,
                                    start=True, stop=True, perf_mode=DR,
                                )
                        est = est_pool.tile([P, 2, QC], BF16, tag="est",
                                            name=f"est_{hp}_{qc}_{j}_{e}")
                        nc.scalar.activation(est[:], stage[:], Exp,
                                             scale=EXP_SCALE, bias=bias_t[:])
                        for ki in range(2):
                            kt = 2 * j + ki
                            nc.tensor.matmul(
                                pacc2[e][:],
                                V_sb[:, kt, h, :],
                                est[:, ki, :],
                                start=(kt == 0), stop=(kt == NT - 1),
                                skip_group_check=True,
                            )
                if qc == 0 and hp + 1 < CS:
                    qk_next = emit_qk_proj(hp + 1)
                if qc == 0 and hp == 0:
                    nc.sync.dma_start(wpb[:], wp_r[:])
                # normalize: reciprocal of the 16*sum(p) denominator row ->
                # partition broadcast -> multiply (16s and e^-4 bias cancel)
                for e, po in enumerate((0, HD)):
                    r32 = recip_pool.tile([1, QC], FP32, tag="recip",
                                          name=f"r32_{hp}_{qc}_{e}")
                    nc.vector.reciprocal(r32[:], pacc2[e][HD:HD + 1, :])
                    rbt = rb_pool.tile([HD, QC], FP32, tag="rb",
                                       name=f"rb_{hp}_{qc}_{e}")
                    nc.gpsimd.partition_broadcast(rbt[:], r32[:])
                    nc.vector.tensor_mul(
                        OT[po:po + HD, hp, qc * QC:(qc + 1) * QC],
                        pacc2[e][0:HD, :], rbt[:],
                    )

        while slot_work:
            slot_work.pop(0)()

        # ---- o-proj tail: cs7 only for nt 4-7 (5-6 ran in hp7/qc0) -------
        for nt in range(4, NT):
            y_sb = ysb_pool.tile([P, DIM], FP32, tag="ysb",
                                 name=f"y_sb2_{nt}")
            py2 = psum.tile([P, 2, QC], FP32, tag="sstage", bufs=2,
                            name=f"py2b_{nt}")
            for fc in range(2):
                nc.tensor.matmul(
                    py2[:, fc, :],
                    OT[:, CS - 1, nt * P:(nt + 1) * P],
                    wpb[:, CS - 1, fc * QC:(fc + 1) * QC],
                    start=True, stop=True, skip_group_check=True,
                )
            nc.vector.tensor_add(
                y_sb[:], py2[:, :, :].rearrange("p a q -> p (a q)"),
                y1[:, nt, :])
            nc.scalar.dma_start(y_r[:, nt, :], y_sb[:])


_CACHE = {}


def _get_nc(N=1024):
    if N not in _CACHE:
        _CACHE[N] = build_nc(N)
    return _CACHE[N]


def _hilo(t):
    hi = t.astype(F8NP)
    lo = (t - hi.astype(np.float32)).astype(F8NP)
    return np.ascontiguousarray(hi), np.ascontiguousarray(lo)


def kernel(x, w_qkv, w_proj, b_proj):
    """Full inputs in, full output out. Shards batch across 8 cores."""
    from concourse.bass_utils import run_bass_kernel_spmd

    B, N, C = x.shape
    assert (B, C) == (8, DIM)
    nc, nm = _get_nc(N)
    x = np.asarray(x, dtype=np.float32)
    wh, wl = _hilo(np.asarray(w_qkv, dtype=np.float32) * 16.0)
    # permute w_proj rows c = d*16+h -> c' = h*64+d to undo the reference's
    # [B, N, hd, H] output interleave (our O^T rows are c' = h*64+d)
    wpp = np.ascontiguousarray(
        np.asarray(w_proj, dtype=np.float32)
        .reshape(HD, H, DIM).transpose(1, 0, 2).reshape(DIM, DIM)
        .astype(BF16NP))
    b_proj_np = np.asarray(b_proj, dtype=np.float32).reshape(DIM)
    in_maps = []
    for b in range(B):
        xh, xl = _hilo(np.ascontiguousarray(x[b].T))
        in_maps.append({nm["xh"]: xh, nm["xl"]: xl, nm["wh"]: wh,
                        nm["wl"]: wl, nm["wp"]: wpp})
    res = run_bass_kernel_spmd(nc, in_maps, core_ids=list(range(8)))
    y = np.stack([res.results[b][nm["y"]] for b in range(B)], axis=0)
    if np.any(b_proj_np):
        # exact host-side bias add; no-op for the zero bias this model ships
        y = (y + b_proj_np.reshape(1, 1, DIM)).astype(np.float32)
    return y
